# revision 1
# baseline (speedup 1.0000x reference)
"""Trainium2 Bass kernel for the linear state rollout problem.

reference: s_{t+1} = A s_t + B u_t, resX[:, t, :] = s_t, s_0 = x[:, 0, :]
shapes: x [256, 1024, 64], u [256, 1024, 7], A [64, 64], B [64, 7]

Strategy (per core, data-parallel over batch: 32 batch elems / core):
  - Chunk the 1024-step scan into 32 chunks of K=32 steps.
  - Pass 1: stride-8 scan over all (chunk, batch) columns at once (zero
    initial state) -> per-chunk input response z_c.
  - Phase B: sequential scan over the 32 chunk carries (tiny matmuls):
    S_{c+1} = A^32 S_c + z_c.
  - Pass 2: stride-8 scan re-run with correct initial states, emitting
    every intermediate state; each matmul packs 2 timesteps into a
    128-partition PSUM tile ([s_{t+2q+2}; s_{t+2q+1}]), 4 matmuls per
    8-timestep step (fp32r = full PE rate at N=512).
  SBUF layout: states live on partitions 0-63, u windows on 64-119
  (fp32r matmul outputs must land at PSUM partition 0).
  All A-power / weight matrices are computed on device from A and B.
  Host only does layout shuffles (transpose/reshape) and final assembly.
"""

import numpy as np

import concourse.bass as bass
import concourse.bacc as bacc
import concourse.tile as tile
from concourse import mybir
from concourse.bass_utils import run_bass_kernel_spmd

F32 = mybir.dt.float32
F32R = mybir.dt.float32r
FP16 = mybir.dt.float16

NCORES = 8
BC = 32          # batch per core
T = 1024
L = 64
CH = 7
K = 32           # chunk length
NCHUNK = T // K  # 32
G = 2            # column groups
CPG = NCHUNK // G   # 16 chunks per group
NG = CPG * BC       # 512 columns per group
S = 4            # window steps per chunk (K / 8)
R = 8            # timesteps per window step
NQ = 4           # psum tiles per window step (2 timesteps each)

_NC_CACHE = None
LAST_RESULT = None


def _build_nc():
    nc = bacc.Bacc("TRN2", target_bir_lowering=False, debug=False,
                   num_devices=NCORES)

    uw = {}
    for s in range(S):
        for g in range(G):
            uw[s, g] = nc.dram_tensor(
                f"u_{s}_{g}", [64, NG], FP16, kind="ExternalInput").ap()
    s0 = nc.dram_tensor("s0", [L, BC], F32R, kind="ExternalInput").ap()
    # consts: [A | A^T | Bpad8 | I] column blocks
    consts = nc.dram_tensor("consts", [L, 2 * L + 8 + L], F32,
                            kind="ExternalInput").ap()
    identf = nc.dram_tensor("identf", [L, L], F32, kind="ExternalInput").ap()
    out = nc.dram_tensor(
        "out", [G * S * NQ, 128, NG], mybir.dt.float16,
        kind="ExternalOutput").ap()

    with tile.TileContext(nc) as tc:
        with (
            tc.tile_pool(name="const", bufs=1) as constp,
            tc.tile_pool(name="wts", bufs=1) as wp,
            tc.tile_pool(name="slabs", bufs=1) as slabp,
            tc.tile_pool(name="stage", bufs=1) as stagep,
            tc.tile_pool(name="prep_ps", bufs=2, space="PSUM") as prep_ps,
            tc.tile_pool(name="mm_ps", bufs=6, space="PSUM") as mm_ps,
        ):
            # ---- constants into SBUF (single DMA)
            const_sb = constp.tile([L, 2 * L + 8 + L], F32, tag="consts")
            nc.sync.dma_start(out=const_sb[:], in_=consts)
            identf_sb = constp.tile([L, L], F32, tag="identf")
            nc.sync.dma_start(out=identf_sb[:], in_=identf)
            amat_sb = const_sb[:, 0:L]
            atmat_sb = const_sb[:, L:2 * L]
            bmat_sb = const_sb[:, 2 * L:2 * L + 8]
            ident_sb = const_sb[:, 2 * L + 8:2 * L + 8 + L]

            zeros_sb = constp.tile([128, 128], F32, tag="zeros")
            nc.vector.memset(zeros_sb[:], 0.0)

            # ---- u windows into chain slabs rows 64-119 (+pad to 127)
            slab = {}
            for s in range(S):
                for g in range(G):
                    t_ = slabp.tile([128, NG], FP16, tag=f"slab{s}{g}",
                                    name=f"slab{s}{g}")
                    nc.sync.dma_start(out=t_[64:128, :], in_=uw[s, g])
                    slab[s, g] = t_
            # phase-B chain state (all chunk carries), fp32r for accuracy
            pbS = slabp.tile([64, NCHUNK * BC], F32R, tag="pbS")
            nc.sync.dma_start(out=pbS[0:64, 0:BC], in_=s0)

            # ---- prep: P_p = (A^T)^p, R_p = A^p; depth-minimized chain
            def _pmm(lhsT, rhs, tag, dt_out=F32):
                ps = prep_ps.tile([64, L], F32, tag="prep", name="prep_ps_t")
                nc.tensor.matmul(ps[:, :], lhsT, rhs)
                t_ = wp.tile([64, L], dt_out, tag=tag, name=tag)
                nc.vector.tensor_copy(t_[:], ps[:])
                return t_

            P = {1: atmat_sb}
            Rr = {1: amat_sb}
            # L1
            P[2] = _pmm(amat_sb, P[1], "Pw2")
            Rr[2] = _pmm(atmat_sb, Rr[1], "Rw2")
            # L2
            P[3] = _pmm(amat_sb, P[2], "Pw3")
            P[4] = _pmm(Rr[2], P[2], "Pw4")
            Rr[3] = _pmm(atmat_sb, Rr[2], "Rw3")
            Rr[4] = _pmm(P[2], Rr[2], "Rw4")
            # L3
            P[5] = _pmm(amat_sb, P[4], "Pw5")
            P[6] = _pmm(Rr[2], P[4], "Pw6")
            P[7] = _pmm(Rr[3], P[4], "Pw7")
            P[8] = _pmm(Rr[4], P[4], "Pw8")
            Rr[8] = _pmm(P[4], Rr[4], "Rw8")
            # ---- TBrev: (A^{7-j} B)^T at rows 8j..8j+6 (row 8j+7 zero)
            # 8-row blocks keep fp32r psum column offsets 8-aligned.
            tps = prep_ps.tile([64, 64], F32, tag="prep", name="prep_ps_t")
            for j in range(8):
                pw = 7 - j
                lhsT = ident_sb if pw == 0 else P[pw][:, :]
                nc.tensor.matmul(tps[0:64, 8 * j:8 * j + 8], lhsT,
                                 bmat_sb)
            tbt = wp.tile([64, 64], F32, tag="tbt")
            nc.vector.tensor_copy(tbt[:], tps[:])
            trp = prep_ps.tile([64, L], F32, tag="prep", name="prep_ps_t")
            nc.tensor.matmul(trp[0:64, 0:64], tbt[0:64, 0:64], identf_sb[:, :],
                             is_transpose=True)
            tbrev = wp.tile([64, L], FP16, tag="tbrev")
            nc.vector.tensor_copy(tbrev[:], trp[0:64, :])

            # ---- W1: rows 0-63 (A^8)^T, rows 64-127 u coeffs
            W1 = wp.tile([128, L], FP16, tag="W1")
            nc.vector.tensor_copy(W1[:], zeros_sb[:, 0:L])
            nc.vector.tensor_copy(W1[0:64, :], P[8][:, :])
            nc.sync.dma_start(out=W1[64:128, :], in_=tbrev[0:64, :])

            # ---- W2[q]: out cols 0-63 -> s_{t+2q+2}; 64-127 -> s_{t+2q+1}
            W2 = []
            for q in range(NQ):
                wt = wp.tile([128, 128], FP16, tag=f"W2{q}", name=f"W2{q}")
                nc.vector.tensor_copy(wt[:], zeros_sb[:])
                nc.vector.tensor_copy(wt[0:64, 0:64], P[2 * q + 2][:, :])
                nc.vector.tensor_copy(wt[0:64, 64:128], P[2 * q + 1][:, :])
                nhi = 8 * (2 * q + 2)
                nlo = 8 * (2 * q + 1)
                nc.sync.dma_start(out=wt[64:64 + nhi, 0:64],
                                  in_=tbrev[8 * (6 - 2 * q):8 * (6 - 2 * q) + nhi, :])
                nc.sync.dma_start(out=wt[64:64 + nlo, 64:128],
                                  in_=tbrev[8 * (7 - 2 * q):8 * (7 - 2 * q) + nlo, :])
                W2.append(wt)

            # L4
            P16 = _pmm(Rr[8], P[8], "Pw16")
            R16 = _pmm(P[8], Rr[8], "Rw16")
            # L5
            P32 = _pmm(R16, P16, "Pw32")
            R32 = _pmm(P16, R16, "Rw32")
            # L6
            P64 = _pmm(R32, P32, "Pw64")
            R64 = _pmm(P32, R32, "Rw64")
            # L7
            P128 = _pmm(R64, P64, "Pw128")
            # fp32r-tagged copies for use as fp32r matmul weights
            P32r = wp.tile([64, L], F32R, tag="P32r")
            nc.vector.tensor_copy(P32r[:], P32[:, :])
            P64r = wp.tile([64, L], F32R, tag="P64r")
            nc.vector.tensor_copy(P64r[:], P64[:, :])
            P128r = wp.tile([64, L], F32R, tag="P128r")
            nc.vector.tensor_copy(P128r[:], P128[:, :])

            # ---- pass 1: z_c (zero-state chunk response), fp32r
            zt = {}
            for g in range(G):
                zt[g] = slabp.tile([128, NG], F32R, tag=f"z{g}",
                                   name=f"zt{g}")
            H = NG // 2
            for s in range(S):
                for g in range(G):
                    for h in range(2):
                        cs = slice(h * H, (h + 1) * H)
                        zps = mm_ps.tile([128, H], F32, tag="mmps",
                                         name="zps")
                        if s == 0:
                            nc.tensor.matmul(zps[0:64, :], W1[64:128, :],
                                             slab[0, g][64:128, cs])
                        else:
                            nc.tensor.matmul(zps[0:64, :], W1[:, :],
                                             slab[s, g][:, cs])
                        dst = slab[s + 1, g] if s < S - 1 else zt[g]
                        if (g + h) % 2 == 0:
                            nc.vector.tensor_copy(dst[0:64, cs],
                                                  zps[0:64, :])
                        else:
                            nc.scalar.copy(dst[0:64, cs], zps[0:64, :])

            # ---- phase B precompute: w_c = A^K z_c + z_{c+1},
            #      v_c = A^{2K} w_c + w_{c+2}  (batched over chunk columns)
            w_sb = {}
            for g in range(G):
                w_sb[g] = slabp.tile([64, NG], F32R, tag=f"w{g}",
                                     name=f"wsb{g}")
                wps = mm_ps.tile([128, NG], F32, tag="mmps", name="wps")
                nc.tensor.matmul(wps[0:64, :], P32r[:, :], zt[g][0:64, :])
                nc.vector.tensor_add(w_sb[g][0:64, 0:NG - BC],
                                     wps[0:64, 0:NG - BC],
                                     zt[g][0:64, BC:NG])
                if g == 0:
                    nc.vector.tensor_add(w_sb[0][0:64, NG - BC:NG],
                                         wps[0:64, NG - BC:NG],
                                         zt[1][0:64, 0:BC])
            v_sb = {}
            for g in range(G):
                v_sb[g] = slabp.tile([64, NG], F32R, tag=f"v{g}",
                                     name=f"vsb{g}")
                vps = mm_ps.tile([128, NG], F32, tag="mmps", name="vps")
                nc.tensor.matmul(vps[0:64, :], P64r[:, :], w_sb[g][0:64, :])
                nc.vector.tensor_add(v_sb[g][0:64, 0:NG - 2 * BC],
                                     vps[0:64, 0:NG - 2 * BC],
                                     w_sb[g][0:64, 2 * BC:NG])
                if g == 0:
                    nc.vector.tensor_add(v_sb[0][0:64, NG - 2 * BC:NG],
                                         vps[0:64, NG - 2 * BC:NG],
                                         w_sb[1][0:64, 0:2 * BC])

            def _sslice(c):
                return pbS[0:64, c * BC:(c + 1) * BC]

            def _round(dst_c, src_c, pw, addend):
                bps = mm_ps.tile([64, BC], F32, tag="mmps", name="bps")
                nc.tensor.matmul(bps[:, :], pw[:, :], _sslice(src_c))
                nc.vector.tensor_add(_sslice(dst_c), bps[:, :], addend)

            def phase_b_round(c):
                # S_{c+4} = A^{4K} S_c + v_c
                g, cl = divmod(c, CPG)
                _round(c + 4, c, P128r, v_sb[g][0:64, cl * BC:(cl + 1) * BC])

            # init: S_1, S_2 from S_0; S_3 from S_1
            _round(1, 0, P32r, zt[0][0:64, 0:BC])
            _round(2, 0, P64r, w_sb[0][0:64, 0:BC])
            _round(3, 1, P64r, w_sb[0][0:64, BC:2 * BC])

            def pass2_step(g, m):
                # q order: 3 first so the carry chain advances earliest
                for q in (3, 0, 1, 2):
                    tau = g * S * NQ + m * NQ + q
                    qps = mm_ps.tile([128, NG], F32, tag="mmps", name="qps")
                    nc.tensor.matmul(qps[:, :], W2[q][:, :], slab[m, g][:, :])
                    st = stagep.tile([128, NG], mybir.dt.float16,
                                     tag=f"st{tau}", name=f"st{tau}")
                    if q in (3, 1):
                        nc.scalar.copy(st[:], qps[:])
                    else:
                        nc.vector.tensor_copy(st[:], qps[:])
                    if q == NQ - 1 and m < S - 1:
                        # carry: next step's state, straight from PSUM;
                        # alternate engines so neither queue gets 3 ops/step
                        if m % 2 == 0:
                            nc.vector.tensor_copy(slab[m + 1, g][0:64, :],
                                                  qps[0:64, :])
                        else:
                            nc.scalar.copy(slab[m + 1, g][0:64, :],
                                           qps[0:64, :])
                    nc.sync.dma_start(out=out[tau], in_=st[:])

            # rounds c=0..11 complete S_4..S_15 (group 0)
            for c in range(0, 12):
                phase_b_round(c)
            nc.vector.tensor_copy(slab[0, 0][0:64, :], pbS[0:64, 0:NG])
            emitted_g1_cast = False
            for m in range(S):
                pass2_step(0, m)
                for c in range(12 + 4 * m, min(12 + 4 * (m + 1), NCHUNK - 4)):
                    phase_b_round(c)
                if m == S - 1 and not emitted_g1_cast:
                    nc.scalar.copy(slab[0, 1][0:64, :], pbS[0:64, NG:2 * NG])
                    emitted_g1_cast = True
            for m in range(S):
                pass2_step(1, m)

    nc.compile()
    return nc


def _get_nc():
    global _NC_CACHE
    if _NC_CACHE is None:
        _NC_CACHE = _build_nc()
    return _NC_CACHE


def _build_in_maps(x, u, A, B):
    A = np.asarray(A, np.float32)
    B = np.asarray(B, np.float32)
    x = np.asarray(x, np.float32)
    u = np.asarray(u, np.float32)
    ident = np.eye(L, dtype=np.float32)
    bmat = np.ascontiguousarray(B)
    in_maps = []
    for core in range(NCORES):
        bsl = slice(core * BC, (core + 1) * BC)
        uc = u[bsl]                                  # [32, 1024, 7]
        w = uc.reshape(BC, G, CPG, S, R, CH)         # [b, g, cl, s, r, ch]
        m = {}
        for s in range(S):
            for g in range(G):
                win = w[:, g, :, s, :, :]            # [b, cl, r, ch]
                win = win.transpose(2, 3, 1, 0)      # [r, ch, cl, b]
                buf = np.zeros((R, 8, NG), np.float16)
                buf[:, :CH, :] = win.reshape(R, CH, NG).astype(np.float16)
                buf = buf.reshape(64, NG)
                m[f"u_{s}_{g}"] = buf
        m["s0"] = np.ascontiguousarray(x[bsl, 0, :].T)
        B8 = np.zeros((L, 8), np.float32)
        B8[:, :CH] = B
        m["consts"] = np.concatenate([A, A.T, B8, ident], axis=1)
        m["identf"] = ident
        in_maps.append(m)
    return in_maps


def kernel(x, u, A, B, stepNum):
    global LAST_RESULT
    stepNum = int(stepNum)
    nc = _get_nc()
    in_maps = _build_in_maps(x, u, A, B)
    res = run_bass_kernel_spmd(nc, in_maps, core_ids=list(range(NCORES)))
    LAST_RESULT = res
    out = np.empty((256, T, L), np.float32)
    for core in range(NCORES):
        od = np.asarray(res.results[core]["out"]).astype(np.float32)
        arr = od.reshape(G, S, NQ, 2, L, CPG, BC)    # [g, m, q, rr, l, cl, b]
        # rr=0 (partitions 0-63) holds t-offset 2q+2; rr=1 holds 2q+1.
        arr = arr[:, :, :, ::-1, :, :, :]            # flip rr -> r: 2q+1+r
        arr = arr.transpose(6, 0, 5, 1, 2, 3, 4)     # [b, g, cl, m, q, r, l]
        arr = np.ascontiguousarray(arr).reshape(BC, T, L)
        out[core * BC:(core + 1) * BC, 1:T, :] = arr[:, 0:T - 1, :]
    out[:, 0, :] = np.asarray(x, np.float32)[:, 0, :]
    if stepNum < T:
        out[:, stepNum:, :] = 0.0
    return out



# revision 6
# speedup vs baseline: 1.1760x; 1.1760x over previous
"""Trainium2 Bass kernel for the linear state rollout problem.

reference: s_{t+1} = A s_t + B u_t, resX[:, t, :] = s_t, s_0 = x[:, 0, :]
shapes: x [256, 1024, 64], u [256, 1024, 7], A [64, 64], B [64, 7]

Strategy (per core, data-parallel over batch: 32 batch elems / core):
  - 32 chunks of K=32 steps; column space 1024 = chunk*32 + batch.
  - All weights (A powers, pass-2 emit matrices W2, chunk-response
    matrices Vs, Kogge-Stone powers) precomputed on host in float64.
  - z_c (chunk input response) via 4 accumulating fp16 matmuls per
    column half — no sequential window chain.  s0 contribution
    (A^32 s0) host-computed, added on device.
  - Carries via 5-level Kogge-Stone scan over chunk columns (batched
    [64x64] fp32r matmuls + vector/gpsimd adds).  Chunks 0-15 are
    final after level 3, so pass-2 for the first column half overlaps
    level 4.
  - Pass 2: per 8-step window, 4 fp16 matmuls [128x128]@[128,512] emit
    2 states each (PSUM-emission-rate optimal); psum->sbuf fp16 copies
    split across vector/scalar/gpsimd; one 512KB output DMA per
    window (8 total) to amortize the ~600ns DMA trigger cost.
  - A few warmup matmuls at start ramp the PE DVFS p-state.
"""

import numpy as np

import concourse.bass as bass
import concourse.bacc as bacc
import concourse.tile as tile
from concourse import mybir
from concourse.bass_utils import run_bass_kernel_spmd

F32 = mybir.dt.float32
F32R = mybir.dt.float32r
FP16 = mybir.dt.float16

NCORES = 8
BC = 32          # batch per core
T = 1024
L = 64
CH = 7
NCHUNK = 32
S = 4            # window steps per chunk
R = 8            # timesteps per window
NQ = 4
NCOL = NCHUNK * BC   # 1024
NWARM = 6        # PE warmup matmuls

_NC_CACHE = None
LAST_RESULT = None


# ---------------------------------------------------------------- host math
def _build_weights(A, B):
    A = np.asarray(A, np.float64)
    B = np.asarray(B, np.float64)
    npow = {}

    def Ap(p):
        if p not in npow:
            npow[p] = np.linalg.matrix_power(A, p)
        return npow[p]

    TBrev = np.zeros((64, L), np.float64)
    for j in range(8):
        TBrev[8 * j:8 * j + CH, :] = (Ap(7 - j) @ B).T

    W2 = np.zeros((NQ, 128, 128), np.float64)
    for q in range(NQ):
        phi = 2 * q + 2
        plo = 2 * q + 1
        W2[q, 0:64, 0:64] = Ap(phi).T
        W2[q, 0:64, 64:128] = Ap(plo).T
        W2[q, 64:64 + 8 * phi, 0:64] = \
            TBrev[8 * (6 - 2 * q):8 * (6 - 2 * q) + 8 * phi]
        W2[q, 64:64 + 8 * plo, 64:128] = \
            TBrev[8 * (7 - 2 * q):8 * (7 - 2 * q) + 8 * plo]

    Vs = np.zeros((S, 64, L), np.float64)
    for s in range(S):
        for r in range(R):
            Vs[s, 8 * r:8 * r + CH, :] = (Ap(31 - 8 * s - r) @ B).T

    Q = np.stack([Ap(32 * (1 << j)).T for j in range(5)], axis=0)

    c16 = np.zeros((128, 768), np.float16)
    for q in range(NQ):
        c16[:, 128 * q:128 * (q + 1)] = W2[q].astype(np.float16)
    for s in range(S):
        c16[64:128, 512 + 64 * s:512 + 64 * (s + 1)] = \
            Vs[s].astype(np.float16)

    c32 = np.zeros((64, 320), np.float32)
    for j in range(5):
        c32[:, 64 * j:64 * (j + 1)] = Q[j].astype(np.float32)

    return c16, c32, Ap(32)


def _build_uw(uc):
    """uc [BC, T, CH] f32 -> [4, 64, 1024] fp16 window layout:
    col = c*32 + b, row = 8r + ch, block s, value u[b, 32c + 8s + r, ch]."""
    v = uc.reshape(BC, NCHUNK, S, R, CH).transpose(2, 3, 4, 1, 0)
    buf = np.zeros((S, R, 8, NCHUNK, BC), np.float16)
    buf[:, :, :CH, :, :] = v.astype(np.float16)
    return buf.reshape(S, 64, NCOL)


# ---------------------------------------------------------------- device
def _build_nc():
    nc = bacc.Bacc("TRN2", target_bir_lowering=False, debug=False,
                   num_devices=NCORES)

    uw0 = nc.dram_tensor("uw0", [64, 2048], FP16, kind="ExternalInput").ap()
    uw1 = nc.dram_tensor("uw1", [64, 2048], FP16, kind="ExternalInput").ap()
    c16 = nc.dram_tensor("c16", [128, 768], FP16, kind="ExternalInput").ap()
    c32 = nc.dram_tensor("c32", [64, 320], F32R, kind="ExternalInput").ap()
    inj = nc.dram_tensor("inj", [64, 32], F32R, kind="ExternalInput").ap()
    s0h = nc.dram_tensor("s0h", [64, 32], FP16, kind="ExternalInput").ap()
    out = nc.dram_tensor("out", [2 * S, 128, 2048], FP16,
                         kind="ExternalOutput").ap()

    with tile.TileContext(nc) as tc:
        with (
            tc.tile_pool(name="const", bufs=1) as constp,
            tc.tile_pool(name="slab", bufs=1) as slabp,
            tc.tile_pool(name="zb", bufs=2) as zbp,
            tc.tile_pool(name="stage", bufs=3) as stagep,
            tc.tile_pool(name="ps", bufs=6, space="PSUM") as psp,
            tc.tile_pool(name="warm_ps", bufs=1, space="PSUM") as wpsp,
        ):
            # ---- SBUF tiles
            w16 = constp.tile([128, 768], FP16, tag="w16")
            qb = constp.tile([64, 320], F32R, tag="qb")
            inj_sb = constp.tile([64, 32], F32R, tag="inj")
            warm = constp.tile([64, 512], FP16, tag="warm")
            slabs = slabp.tile([128, 4096], FP16, tag="slabs")
            # ping-pong scan buffers: level j reads ztb[j%2], writes
            # ztb[(j+1)%2] — keeps part-B reads off part-A's fresh writes
            ztb = [slabp.tile([64, NCOL], F32R, tag=f"zt{i}",
                              name=f"zt{i}") for i in range(2)]
            zt = ztb[0]

            # ---- input DMAs (sync: critical path; scalar: the rest)
            nc.sync.dma_start(out=w16[:], in_=c16)
            nc.sync.dma_start(out=slabs[64:128, 0:2048], in_=uw0)
            nc.sync.dma_start(out=slabs[64:128, 2048:4096], in_=uw1)
            nc.scalar.dma_start(out=qb[:], in_=c32)
            nc.scalar.dma_start(out=inj_sb[:], in_=inj)
            nc.scalar.dma_start(out=slabs[0:64, 0:32], in_=s0h)

            # ---- PE warmup (ramps the DVFS p-state before real matmuls)
            nc.vector.memset(warm[:], 0.0)
            for i in range(NWARM):
                wps = wpsp.tile([64, 512], F32, tag="wps", name="wps")
                nc.tensor.matmul(wps[:, :], warm[:, 0:64], warm[:, :])

            # ---- z: chunk responses, 4 accumulating matmuls per half
            psz = {}
            for h in range(2):
                psz[h] = psp.tile([64, 512], F32, tag="ps", name=f"psz{h}")
                for s in range(S):
                    nc.tensor.matmul(
                        psz[h][:, :],
                        w16[64:128, 512 + 64 * s:512 + 64 * (s + 1)],
                        slabs[64:128, s * 1024 + h * 512:s * 1024 + h * 512 + 512],
                        start=(s == 0), stop=(s == S - 1))
            nc.vector.tensor_copy(zt[:, 0:512], psz[0][:, :])
            nc.vector.tensor_add(zt[:, 0:32], zt[:, 0:32], inj_sb[:, :])
            nc.scalar.copy(zt[:, 512:1024], psz[1][:, :])

            # ---- Kogge-Stone over chunks.  Part A (cols < 512, the H0
            # prefix) chains on vector; part B bounces through scalar to a
            # SBUF scratch so gpsimd can do the add, keeping the critical
            # A-chain latency low.
            def ks_level(j):
                d0 = 32 * (1 << j)
                rd, wr = ztb[j % 2], ztb[(j + 1) % 2]
                cw = 512 - d0
                psa = psp.tile([64, 512], F32, tag="ps", name=f"ksa{j}")
                nc.tensor.matmul(psa[:, 0:cw], qb[:, 64 * j:64 * (j + 1)],
                                 rd[:, 0:cw])
                psb = psp.tile([64, 512], F32, tag="ps", name=f"ksb{j}")
                nc.tensor.matmul(psb[:, :], qb[:, 64 * j:64 * (j + 1)],
                                 rd[:, 512 - d0:1024 - d0])
                nc.vector.tensor_add(wr[:, d0:512], psa[:, 0:cw],
                                     rd[:, d0:512])
                nc.vector.tensor_copy(wr[:, 0:d0], rd[:, 0:d0])
                zb = zbp.tile([64, 512], F32, tag="zb", name=f"zb{j}")
                nc.scalar.copy(zb[:], psb[:, :])
                nc.gpsimd.tensor_add(wr[:, 512:1024], zb[:],
                                     rd[:, 512:1024])

            for j in range(4):
                ks_level(j)
            # after 4 levels the final buffer is ztb[0] (= zt) again

            # ---- pass 2
            def pass2_step(h, m):
                st = stagep.tile([128, 2048], FP16, tag="st",
                                 name=f"st{h}{m}")
                base = m * 1024 + h * 512
                for q in (3, 0, 1, 2):
                    ps = psp.tile([128, 512], F32, tag="ps",
                                  name=f"q{h}{m}{q}")
                    nc.tensor.matmul(ps[:, :],
                                     w16[:, 128 * q:128 * (q + 1)],
                                     slabs[:, base:base + 512])
                    d = q * 512
                    if q == 3:
                        if m < S - 1:
                            nb = (m + 1) * 1024 + h * 512
                            nc.vector.tensor_copy(
                                slabs[0:64, nb:nb + 512], ps[0:64, :])
                            nc.scalar.copy(st[:, d:d + 512], ps[:, :])
                        else:
                            nc.vector.tensor_copy(st[:, d:d + 256],
                                                  ps[:, 0:256])
                            nc.scalar.copy(st[:, d + 256:d + 512],
                                           ps[:, 256:512])
                    elif q == 1:
                        nc.vector.tensor_copy(st[:, d:d + 512], ps[:, :])
                    else:  # q == 0, 2
                        nc.scalar.copy(st[:, d:d + 512], ps[:, :])
                nc.sync.dma_start(out=out[h * S + m], in_=st[:])

            # H0 carries final after level-3 part A
            nc.gpsimd.tensor_copy(slabs[0:64, 32:512], zt[:, 0:480])
            pass2_step(0, 0)
            # level 4 (finalizes H1 carries) overlaps pass-2 H0
            ps4 = psp.tile([64, 512], F32, tag="ps", name="ks4")
            nc.tensor.matmul(ps4[:, :], qb[:, 256:320], zt[:, 0:512])
            nc.vector.tensor_add(zt[:, 512:1024], ps4[:, :],
                                 zt[:, 512:1024])
            pass2_step(0, 1)
            nc.gpsimd.tensor_copy(slabs[0:64, 512:1024], zt[:, 480:992])
            pass2_step(0, 2)
            pass2_step(0, 3)
            for m in range(S):
                pass2_step(1, m)

    nc.compile()
    return nc


def _get_nc():
    global _NC_CACHE
    if _NC_CACHE is None:
        _NC_CACHE = _build_nc()
    return _NC_CACHE


def _build_in_maps(x, u, A, B):
    x = np.asarray(x, np.float32)
    u = np.asarray(u, np.float32)
    c16, c32, A32 = _build_weights(A, B)
    in_maps = []
    for core in range(NCORES):
        bsl = slice(core * BC, (core + 1) * BC)
        uwS = _build_uw(u[bsl])
        s0 = x[bsl, 0, :].T                       # [64, 32] f32
        m = {
            "uw0": np.ascontiguousarray(
                np.concatenate([uwS[0], uwS[1]], axis=1)),
            "uw1": np.ascontiguousarray(
                np.concatenate([uwS[2], uwS[3]], axis=1)),
            "c16": c16,
            "c32": c32,
            "inj": (A32 @ s0.astype(np.float64)).astype(np.float32),
            "s0h": s0.astype(np.float16),
        }
        in_maps.append(m)
    return in_maps


def kernel(x, u, A, B, stepNum):
    global LAST_RESULT
    stepNum = int(stepNum)
    nc = _get_nc()
    in_maps = _build_in_maps(x, u, A, B)
    res = run_bass_kernel_spmd(nc, in_maps, core_ids=list(range(NCORES)))
    LAST_RESULT = res
    out = np.empty((NCORES * BC, T, L), np.float32)
    for core in range(NCORES):
        od = np.asarray(res.results[core]["out"]).astype(np.float32)
        arr = od.reshape(2, S, 2, L, NQ, 16, BC)   # [h, m, rr, l, q, cl, b]
        arr = arr[:, :, ::-1]                      # k: 0 -> 2q+1, 1 -> 2q+2
        arr = arr.transpose(6, 0, 5, 1, 4, 2, 3)   # [b, h, cl, m, q, k, l]
        arr = np.ascontiguousarray(arr).reshape(BC, T, L)
        out[core * BC:(core + 1) * BC, 1:T, :] = arr[:, 0:T - 1, :]
    out[:, 0, :] = np.asarray(x, np.float32)[:, 0, :]
    if stepNum < T:
        out[:, stepNum:, :] = 0.0
    return out


# revision 11
# speedup vs baseline: 1.2002x; 1.0206x over previous
"""Trainium2 Bass kernel for the linear state rollout problem.

reference: s_{t+1} = A s_t + B u_t, resX[:, t, :] = s_t, s_0 = x[:, 0, :]
shapes: x [256, 1024, 64], u [256, 1024, 7], A [64, 64], B [64, 7]

Strategy (per core, data-parallel over batch: 32 batch elems / core):
  - 32 chunks of K=32 steps; column space 1024 = chunk*32 + batch.
  - All weights (A powers, pass-2 emit matrices W2, chunk-response
    matrices Vs, Kogge-Stone powers) precomputed on host in float64.
  - z_c (chunk input response) via 4 accumulating fp16 matmuls per
    column half — no sequential window chain.  s0 contribution
    (A^32 s0) host-computed, added on device.
  - Carries via 5-level Kogge-Stone scan over chunk columns (batched
    [64x64] fp32r matmuls + vector/gpsimd adds).  Chunks 0-15 are
    final after level 3, so pass-2 for the first column half overlaps
    level 4.
  - Pass 2: per 8-step window, 4 fp16 matmuls [128x128]@[128,512] emit
    2 states each (PSUM-emission-rate optimal); psum->sbuf fp16 copies
    split across vector/scalar/gpsimd; one 512KB output DMA per
    window (8 total) to amortize the ~600ns DMA trigger cost.
  - A few warmup matmuls at start ramp the PE DVFS p-state.
"""

import numpy as np

import concourse.bass as bass
import concourse.bacc as bacc
import concourse.tile as tile
from concourse import mybir
from concourse.bass_utils import run_bass_kernel_spmd

F32 = mybir.dt.float32
F32R = mybir.dt.float32r
FP16 = mybir.dt.float16

NCORES = 8
BC = 32          # batch per core
T = 1024
L = 64
CH = 7
NCHUNK = 32
S = 4            # window steps per chunk
R = 8            # timesteps per window
NQ = 4
NCOL = NCHUNK * BC   # 1024
NWARM = 6        # PE warmup matmuls

_NC_CACHE = None
LAST_RESULT = None


# ---------------------------------------------------------------- host math
def _build_weights(A, B):
    A = np.asarray(A, np.float64)
    B = np.asarray(B, np.float64)
    npow = {}

    def Ap(p):
        if p not in npow:
            npow[p] = np.linalg.matrix_power(A, p)
        return npow[p]

    TBrev = np.zeros((64, L), np.float64)
    for j in range(8):
        TBrev[8 * j:8 * j + CH, :] = (Ap(7 - j) @ B).T

    W2 = np.zeros((NQ, 128, 128), np.float64)
    for q in range(NQ):
        phi = 2 * q + 2
        plo = 2 * q + 1
        W2[q, 0:64, 0:64] = Ap(phi).T
        W2[q, 0:64, 64:128] = Ap(plo).T
        W2[q, 64:64 + 8 * phi, 0:64] = \
            TBrev[8 * (6 - 2 * q):8 * (6 - 2 * q) + 8 * phi]
        W2[q, 64:64 + 8 * plo, 64:128] = \
            TBrev[8 * (7 - 2 * q):8 * (7 - 2 * q) + 8 * plo]

    Vs = np.zeros((S, 64, L), np.float64)
    for s in range(S):
        for r in range(R):
            Vs[s, 8 * r:8 * r + CH, :] = (Ap(31 - 8 * s - r) @ B).T

    Q = np.stack([Ap(32 * (1 << j)).T for j in range(5)], axis=0)

    cz = np.zeros((64, 256), np.float16)
    for s in range(S):
        cz[:, 64 * s:64 * (s + 1)] = Vs[s].astype(np.float16)

    cw2 = np.zeros((128, 512), np.float16)
    for q in range(NQ):
        cw2[:, 128 * q:128 * (q + 1)] = W2[q].astype(np.float16)

    c32 = np.zeros((64, 320), np.float32)
    for j in range(5):
        c32[:, 64 * j:64 * (j + 1)] = Q[j].astype(np.float32)

    return cz, cw2, c32, Ap(32)


def _build_uw(uc):
    """uc [BC, T, CH] f32 -> [4, 64, 1024] fp16 window layout:
    col = c*32 + b, row = 8r + ch, block s, value u[b, 32c + 8s + r, ch]."""
    v = uc.reshape(BC, NCHUNK, S, R, CH).transpose(2, 3, 4, 1, 0)
    buf = np.zeros((S, R, 8, NCHUNK, BC), np.float16)
    buf[:, :, :CH, :, :] = v.astype(np.float16)
    return buf.reshape(S, 64, NCOL)


# ---------------------------------------------------------------- device
def _build_nc():
    nc = bacc.Bacc("TRN2", target_bir_lowering=False, debug=False,
                   num_devices=NCORES)

    uwd = {}
    for s in range(S):
        for h in range(2):
            uwd[s, h] = nc.dram_tensor(f"uw{s}{h}", [64, 512], FP16,
                                       kind="ExternalInput").ap()
    cz = nc.dram_tensor("cz", [64, 256], FP16, kind="ExternalInput").ap()
    cw2 = nc.dram_tensor("cw2", [128, 512], FP16, kind="ExternalInput").ap()
    c32 = nc.dram_tensor("c32", [64, 320], F32R, kind="ExternalInput").ap()
    inj = nc.dram_tensor("inj", [64, 32], F32R, kind="ExternalInput").ap()
    s0h = nc.dram_tensor("s0h", [64, 32], FP16, kind="ExternalInput").ap()
    out = nc.dram_tensor("out", [2 * S, 128, 2048], FP16,
                         kind="ExternalOutput").ap()

    with tile.TileContext(nc) as tc:
        with (
            tc.tile_pool(name="const", bufs=1) as constp,
            tc.tile_pool(name="slab", bufs=1) as slabp,
            tc.tile_pool(name="zb", bufs=2) as zbp,
            tc.tile_pool(name="stage", bufs=3) as stagep,
            tc.tile_pool(name="ps", bufs=6, space="PSUM") as psp,
            tc.tile_pool(name="warm_ps", bufs=1, space="PSUM") as wpsp,
        ):
            # ---- SBUF tiles
            zw = constp.tile([128, 256], FP16, tag="zw")
            w2 = constp.tile([128, 512], FP16, tag="w2")
            qb = constp.tile([64, 320], F32R, tag="qb")
            inj_sb = constp.tile([64, 32], F32R, tag="inj")
            warm = constp.tile([64, 512], FP16, tag="warm")
            slabs = slabp.tile([128, 4096], FP16, tag="slabs")
            zt = slabp.tile([64, NCOL], F32R, tag="zt")

            # ---- input DMAs.  sync: u windows in z-consumption order;
            # scalar: weights (Vs first — it gates the z matmuls).
            nc.scalar.dma_start(out=zw[64:128, :], in_=cz)
            for s in range(S):
                nc.sync.dma_start(
                    out=slabs[64:128, s * 1024:s * 1024 + 512],
                    in_=uwd[s, 0])
            for s in range(S):
                nc.sync.dma_start(
                    out=slabs[64:128, s * 1024 + 512:s * 1024 + 1024],
                    in_=uwd[s, 1])
            nc.scalar.dma_start(out=qb[:], in_=c32)
            nc.scalar.dma_start(out=w2[:], in_=cw2)
            nc.scalar.dma_start(out=inj_sb[:], in_=inj)
            nc.scalar.dma_start(out=slabs[0:64, 0:32], in_=s0h)

            # ---- PE warmup (keeps the DVFS p-state ramp alive)
            nc.vector.memset(warm[:], 0.0)

            def junk_mm(i):
                wps = wpsp.tile([64, 512], F32, tag="wps", name="wps")
                nc.tensor.matmul(wps[:, :], warm[:, 0:64], warm[:, :])

            junk_mm(0)
            junk_mm(1)

            # ---- z: chunk responses, 4 accumulating matmuls per half
            psz = {}
            for h in range(2):
                psz[h] = psp.tile([64, 512], F32, tag="ps", name=f"psz{h}")
                for s in range(S):
                    nc.tensor.matmul(
                        psz[h][:, :],
                        zw[64:128, 64 * s:64 * (s + 1)],
                        slabs[64:128, s * 1024 + h * 512:s * 1024 + h * 512 + 512],
                        start=(s == 0), stop=(s == S - 1))
            nc.vector.tensor_copy(zt[:, 0:512], psz[0][:, :])
            nc.vector.tensor_add(zt[:, 0:32], zt[:, 0:32], inj_sb[:, :])
            nc.scalar.copy(zt[:, 512:1024], psz[1][:, :])

            # ---- Kogge-Stone over chunks, in place.  mmB is emitted
            # before addA so the WAR dependency makes addA (which
            # overwrites boundary cols mmB reads) wait for mmB.
            def ks_level(j):
                d0 = 32 * (1 << j)
                cw = 512 - d0
                psa = psp.tile([64, 512], F32, tag="ps", name=f"ksa{j}")
                nc.tensor.matmul(psa[:, 0:cw], qb[:, 64 * j:64 * (j + 1)],
                                 zt[:, 0:cw])
                psb = psp.tile([64, 512], F32, tag="ps", name=f"ksb{j}")
                nc.tensor.matmul(psb[:, :], qb[:, 64 * j:64 * (j + 1)],
                                 zt[:, 512 - d0:1024 - d0])
                junk_mm(10 + j)
                nc.vector.tensor_add(zt[:, d0:512], psa[:, 0:cw],
                                     zt[:, d0:512])
                nc.vector.tensor_add(zt[:, 512:1024], psb[:, :],
                                     zt[:, 512:1024])

            for j in range(4):
                ks_level(j)

            # ---- pass 2
            def pass2_step(h, m):
                st = stagep.tile([128, 2048], FP16, tag="st",
                                 name=f"st{h}{m}")
                base = m * 1024 + h * 512
                for q in (3, 0, 1, 2):
                    ps = psp.tile([128, 512], F32, tag="ps",
                                  name=f"q{h}{m}{q}")
                    nc.tensor.matmul(ps[:, :],
                                     w2[:, 128 * q:128 * (q + 1)],
                                     slabs[:, base:base + 512])
                    d = q * 512
                    if q == 3:
                        if m < S - 1:
                            nb = (m + 1) * 1024 + h * 512
                            nc.vector.tensor_copy(
                                slabs[0:64, nb:nb + 512], ps[0:64, :])
                            nc.scalar.copy(st[:, d:d + 512], ps[:, :])
                        else:
                            nc.scalar.copy(st[:, d:d + 512], ps[:, :])
                    elif q == 1:
                        nc.vector.tensor_copy(st[:, d:d + 512], ps[:, :])
                    elif q == 0:
                        nc.scalar.copy(st[:, d:d + 512], ps[:, :])
                    else:  # q == 2: split across both engines
                        if m < S - 1:
                            nc.vector.tensor_copy(st[:, d:d + 256],
                                                  ps[:, 0:256])
                            nc.scalar.copy(st[:, d + 256:d + 512],
                                           ps[:, 256:512])
                        else:
                            nc.vector.tensor_copy(st[:, d:d + 512],
                                                  ps[:, :])
                nc.sync.dma_start(out=out[h * S + m], in_=st[:])

            # H0 carries final after level-3 part A
            nc.vector.tensor_copy(slabs[0:64, 32:512], zt[:, 0:480])
            pass2_step(0, 0)
            # level 4 (finalizes H1 carries) overlaps pass-2 H0
            ps4 = psp.tile([64, 512], F32, tag="ps", name="ks4")
            nc.tensor.matmul(ps4[:, :], qb[:, 256:320], zt[:, 0:512])
            nc.vector.tensor_add(zt[:, 512:1024], ps4[:, :],
                                 zt[:, 512:1024])
            pass2_step(0, 1)
            nc.gpsimd.tensor_copy(slabs[0:64, 512:1024], zt[:, 480:992])
            pass2_step(0, 2)
            pass2_step(0, 3)
            for m in range(S):
                pass2_step(1, m)

    nc.compile()
    return nc


def _get_nc():
    global _NC_CACHE
    if _NC_CACHE is None:
        _NC_CACHE = _build_nc()
    return _NC_CACHE


def _build_in_maps(x, u, A, B):
    x = np.asarray(x, np.float32)
    u = np.asarray(u, np.float32)
    cz, cw2, c32, A32 = _build_weights(A, B)
    in_maps = []
    for core in range(NCORES):
        bsl = slice(core * BC, (core + 1) * BC)
        uwS = _build_uw(u[bsl])
        s0 = x[bsl, 0, :].T                       # [64, 32] f32
        m = {
            "cz": cz,
            "cw2": cw2,
            "c32": c32,
            "inj": (A32 @ s0.astype(np.float64)).astype(np.float32),
            "s0h": s0.astype(np.float16),
        }
        for s in range(S):
            for h in range(2):
                m[f"uw{s}{h}"] = np.ascontiguousarray(
                    uwS[s, :, h * 512:(h + 1) * 512])
        in_maps.append(m)
    return in_maps


def kernel(x, u, A, B, stepNum):
    global LAST_RESULT
    stepNum = int(stepNum)
    nc = _get_nc()
    in_maps = _build_in_maps(x, u, A, B)
    res = run_bass_kernel_spmd(nc, in_maps, core_ids=list(range(NCORES)))
    LAST_RESULT = res
    out = np.empty((NCORES * BC, T, L), np.float32)
    for core in range(NCORES):
        od = np.asarray(res.results[core]["out"]).astype(np.float32)
        arr = od.reshape(2, S, 2, L, NQ, 16, BC)   # [h, m, rr, l, q, cl, b]
        arr = arr[:, :, ::-1]                      # k: 0 -> 2q+1, 1 -> 2q+2
        arr = arr.transpose(6, 0, 5, 1, 4, 2, 3)   # [b, h, cl, m, q, k, l]
        arr = np.ascontiguousarray(arr).reshape(BC, T, L)
        out[core * BC:(core + 1) * BC, 1:T, :] = arr[:, 0:T - 1, :]
    out[:, 0, :] = np.asarray(x, np.float32)[:, 0, :]
    if stepNum < T:
        out[:, stepNum:, :] = 0.0
    return out


# revision 16
# speedup vs baseline: 1.2781x; 1.0649x over previous
"""Trainium2 Bass kernel for the linear state rollout problem.

reference: s_{t+1} = A s_t + B u_t, resX[:, t, :] = s_t, s_0 = x[:, 0, :]
shapes: x [256, 1024, 64], u [256, 1024, 7], A [64, 64], B [64, 7]

Strategy (per core, data-parallel over batch: 32 batch elems / core):
  - 32 chunks of K=32 steps; column space 1024 = chunk*32 + batch.
  - All weights (A powers, pass-2 emit matrices W2, chunk-response
    matrices Vs, Kogge-Stone powers) precomputed on host in float64.
  - z_c (chunk input response) via 4 accumulating fp16 matmuls per
    column half — no sequential window chain.  s0 contribution
    (A^32 s0) host-computed, added on device.
  - Carries via 5-level Kogge-Stone scan over chunk columns (batched
    [64x64] fp32r matmuls + vector/gpsimd adds).  Chunks 0-15 are
    final after level 3, so pass-2 for the first column half overlaps
    level 4.
  - Pass 2: per 8-step window, 4 fp16 matmuls [128x128]@[128,512] emit
    2 states each (PSUM-emission-rate optimal); psum->sbuf fp16 copies
    split across vector/scalar/gpsimd; one 512KB output DMA per
    window (8 total) to amortize the ~600ns DMA trigger cost.
  - A few warmup matmuls at start ramp the PE DVFS p-state.
"""

import numpy as np

import concourse.bass as bass
import concourse.bacc as bacc
import concourse.tile as tile
from concourse import mybir
from concourse.bass_utils import run_bass_kernel_spmd

F32 = mybir.dt.float32
F32R = mybir.dt.float32r
FP16 = mybir.dt.float16

NCORES = 8
BC = 32          # batch per core
T = 1024
L = 64
CH = 7
NCHUNK = 32
S = 4            # window steps per chunk
R = 8            # timesteps per window
NQ = 4
NCOL = NCHUNK * BC   # 1024
NWARM = 6        # PE warmup matmuls

_NC_CACHE = None
LAST_RESULT = None


# ---------------------------------------------------------------- host math
def _build_weights(A, B):
    A = np.asarray(A, np.float64)
    B = np.asarray(B, np.float64)
    npow = {}

    def Ap(p):
        if p not in npow:
            npow[p] = np.linalg.matrix_power(A, p)
        return npow[p]

    TBrev = np.zeros((64, L), np.float64)
    for j in range(8):
        TBrev[8 * j:8 * j + CH, :] = (Ap(7 - j) @ B).T

    W2 = np.zeros((NQ, 128, 128), np.float64)
    for q in range(NQ):
        phi = 2 * q + 2
        plo = 2 * q + 1
        W2[q, 0:64, 0:64] = Ap(phi).T
        W2[q, 0:64, 64:128] = Ap(plo).T
        W2[q, 64:64 + 8 * phi, 0:64] = \
            TBrev[8 * (6 - 2 * q):8 * (6 - 2 * q) + 8 * phi]
        W2[q, 64:64 + 8 * plo, 64:128] = \
            TBrev[8 * (7 - 2 * q):8 * (7 - 2 * q) + 8 * plo]

    Vs = np.zeros((S, 64, L), np.float64)
    for s in range(S):
        for r in range(R):
            Vs[s, 8 * r:8 * r + CH, :] = (Ap(31 - 8 * s - r) @ B).T

    Q = np.stack([Ap(32 * (1 << j)).T for j in range(5)], axis=0)

    cz = np.zeros((64, 256), np.float16)
    for s in range(S):
        cz[:, 64 * s:64 * (s + 1)] = Vs[s].astype(np.float16)

    cw2 = np.zeros((128, 512), np.float16)
    for q in range(NQ):
        cw2[:, 128 * q:128 * (q + 1)] = W2[q].astype(np.float16)

    c32 = np.zeros((64, 320), np.float32)
    for j in range(5):
        c32[:, 64 * j:64 * (j + 1)] = Q[j].astype(np.float32)

    return cz, cw2, c32, Ap(32)


def _build_uw(uc):
    """uc [BC, T, CH] f32 -> [4, 64, 1024] fp16 window layout:
    col = c*32 + b, row = 8r + ch, block s, value u[b, 32c + 8s + r, ch]."""
    v = uc.reshape(BC, NCHUNK, S, R, CH).transpose(2, 3, 4, 1, 0)
    buf = np.zeros((S, R, 8, NCHUNK, BC), np.float16)
    buf[:, :, :CH, :, :] = v.astype(np.float16)
    return buf.reshape(S, 64, NCOL)


# ---------------------------------------------------------------- device
def _build_nc():
    nc = bacc.Bacc("TRN2", target_bir_lowering=False, debug=False,
                   num_devices=NCORES)

    uwa = nc.dram_tensor("uwa", [64, 2048], FP16, kind="ExternalInput").ap()
    uwb = nc.dram_tensor("uwb", [64, 2048], FP16, kind="ExternalInput").ap()
    cz = nc.dram_tensor("cz", [64, 256], FP16, kind="ExternalInput").ap()
    cw2 = nc.dram_tensor("cw2", [128, 512], FP16, kind="ExternalInput").ap()
    c32 = nc.dram_tensor("c32", [64, 320], F32R, kind="ExternalInput").ap()
    inj = nc.dram_tensor("inj", [64, 32], F32R, kind="ExternalInput").ap()
    s0h = nc.dram_tensor("s0h", [64, 32], FP16, kind="ExternalInput").ap()
    out = nc.dram_tensor("out", [2 * S, 128, 2048], FP16,
                         kind="ExternalOutput").ap()

    with tile.TileContext(nc) as tc:
        with (
            tc.tile_pool(name="const", bufs=1) as constp,
            tc.tile_pool(name="slab", bufs=1) as slabp,
            tc.tile_pool(name="zb", bufs=2) as zbp,
            tc.tile_pool(name="stage", bufs=3) as stagep,
            tc.tile_pool(name="ps", bufs=6, space="PSUM") as psp,
            tc.tile_pool(name="warm_ps", bufs=1, space="PSUM") as wpsp,
        ):
            # ---- SBUF tiles
            zw = constp.tile([128, 256], FP16, tag="zw")
            w2 = constp.tile([128, 512], FP16, tag="w2")
            qb = constp.tile([64, 320], F32R, tag="qb")
            inj_sb = constp.tile([64, 32], F32R, tag="inj")
            warm = constp.tile([64, 512], FP16, tag="warm")
            slabs = slabp.tile([128, 4096], FP16, tag="slabs")
            zt = slabp.tile([64, NCOL], F32R, tag="zt")

            # ---- input DMAs.  Slab layout is h-major: block (h, s) at
            # cols h*2048 + s*512, so each half's u windows land in one
            # contiguous DMA.  scalar: weights (Vs first — gates z mms).
            nc.scalar.dma_start(out=zw[64:128, :], in_=cz)
            nc.sync.dma_start(out=slabs[64:128, 0:2048], in_=uwa)
            nc.sync.dma_start(out=slabs[64:128, 2048:4096], in_=uwb)
            nc.scalar.dma_start(out=qb[:], in_=c32)
            nc.scalar.dma_start(out=w2[:], in_=cw2)
            nc.scalar.dma_start(out=inj_sb[:], in_=inj)
            nc.scalar.dma_start(out=slabs[0:64, 0:32], in_=s0h)

            # ---- PE warmup (keeps the DVFS p-state ramp alive)
            nc.vector.memset(warm[:], 0.0)

            def junk_mm(i):
                wps = wpsp.tile([64, 512], F32, tag="wps", name="wps")
                nc.tensor.matmul(wps[:, :], warm[:, 0:64], warm[:, :])

            junk_mm(0)
            junk_mm(1)

            # ---- z: chunk responses, 4 accumulating matmuls per half
            psz = {}
            for h in range(2):
                psz[h] = psp.tile([64, 512], F32, tag="ps", name=f"psz{h}")
                for s in range(S):
                    nc.tensor.matmul(
                        psz[h][:, :],
                        zw[64:128, 64 * s:64 * (s + 1)],
                        slabs[64:128, h * 2048 + s * 512:h * 2048 + s * 512 + 512],
                        start=(s == 0), stop=(s == S - 1))
            nc.vector.tensor_copy(zt[:, 0:512], psz[0][:, :])
            nc.vector.tensor_add(zt[:, 0:32], zt[:, 0:32], inj_sb[:, :])
            nc.scalar.copy(zt[:, 512:1024], psz[1][:, :])
            junk_mm(8)

            # ---- Kogge-Stone over chunks, in place.  addA is split into
            # a main part (cols mmB never reads — keeps the A-chain
            # decoupled from the B-chain) and a boundary part that must
            # wait for mmB's read of the old values (WAR).
            def ks_level(j):
                d0 = 32 * (1 << j)
                cw = 512 - d0
                bnd = max(d0, 512 - d0)   # boundary start: [bnd, 512)
                psa = psp.tile([64, 512], F32, tag="ps", name=f"ksa{j}")
                nc.tensor.matmul(psa[:, 0:cw], qb[:, 64 * j:64 * (j + 1)],
                                 zt[:, 0:cw])
                psb = psp.tile([64, 512], F32, tag="ps", name=f"ksb{j}")
                nc.tensor.matmul(psb[:, :], qb[:, 64 * j:64 * (j + 1)],
                                 zt[:, 512 - d0:1024 - d0])
                junk_mm(10 + j)
                if bnd > d0:
                    nc.vector.tensor_add(zt[:, d0:bnd],
                                         psa[:, 0:bnd - d0],
                                         zt[:, d0:bnd])
                nc.vector.tensor_add(zt[:, bnd:512],
                                     psa[:, bnd - d0:cw],
                                     zt[:, bnd:512])
                nc.vector.tensor_add(zt[:, 512:1024], psb[:, :],
                                     zt[:, 512:1024])

            for j in range(4):
                ks_level(j)

            # ---- pass 2
            def pass2_step(h, m):
                st = stagep.tile([128, 2048], FP16, tag="st",
                                 name=f"st{h}{m}")
                base = h * 2048 + m * 512
                for q in (3, 0, 1, 2):
                    ps = psp.tile([128, 512], F32, tag="ps",
                                  name=f"q{h}{m}{q}")
                    nc.tensor.matmul(ps[:, :],
                                     w2[:, 128 * q:128 * (q + 1)],
                                     slabs[:, base:base + 512])
                    d = q * 512
                    if q == 3:
                        if m < S - 1:
                            nb = base + 512
                            nc.vector.tensor_copy(
                                slabs[0:64, nb:nb + 512], ps[0:64, :])
                            nc.scalar.copy(st[:, d:d + 512], ps[:, :])
                        else:
                            nc.scalar.copy(st[:, d:d + 512], ps[:, :])
                    elif q == 1:
                        nc.vector.tensor_copy(st[:, d:d + 512], ps[:, :])
                    elif q == 0:
                        nc.scalar.copy(st[:, d:d + 512], ps[:, :])
                    else:  # q == 2: split across both engines
                        if m < S - 1:
                            nc.vector.tensor_copy(st[:, d:d + 256],
                                                  ps[:, 0:256])
                            nc.scalar.copy(st[:, d + 256:d + 512],
                                           ps[:, 256:512])
                        else:
                            nc.vector.tensor_copy(st[:, d:d + 512],
                                                  ps[:, :])
                w = h * S + m
                if w == 2 * S - 1:
                    # split the last DMA so the tail transfer is shorter
                    nc.sync.dma_start(out=out[w, :, 0:1024],
                                      in_=st[:, 0:1024])
                    nc.sync.dma_start(out=out[w, :, 1024:2048],
                                      in_=st[:, 1024:2048])
                else:
                    nc.sync.dma_start(out=out[w], in_=st[:])

            # H0 carries final after level 3
            nc.vector.tensor_copy(slabs[0:64, 32:512], zt[:, 0:480])
            pass2_step(0, 0)
            # level 4 (finalizes H1 carries) overlaps pass-2 H0
            ps4 = psp.tile([64, 512], F32, tag="ps", name="ks4")
            nc.tensor.matmul(ps4[:, :], qb[:, 256:320], zt[:, 0:512])
            nc.vector.tensor_add(zt[:, 512:1024], ps4[:, :],
                                 zt[:, 512:1024])
            pass2_step(0, 1)
            nc.gpsimd.tensor_copy(slabs[0:64, 2048:2560], zt[:, 480:992])
            pass2_step(0, 2)
            pass2_step(0, 3)
            for m in range(S):
                pass2_step(1, m)

    nc.compile()
    return nc


def _get_nc():
    global _NC_CACHE
    if _NC_CACHE is None:
        _NC_CACHE = _build_nc()
    return _NC_CACHE


def _build_in_maps(x, u, A, B):
    x = np.asarray(x, np.float32)
    u = np.asarray(u, np.float32)
    cz, cw2, c32, A32 = _build_weights(A, B)
    in_maps = []
    for core in range(NCORES):
        bsl = slice(core * BC, (core + 1) * BC)
        uwS = _build_uw(u[bsl])
        s0 = x[bsl, 0, :].T                       # [64, 32] f32
        m = {
            "cz": cz,
            "cw2": cw2,
            "c32": c32,
            "inj": (A32 @ s0.astype(np.float64)).astype(np.float32),
            "s0h": s0.astype(np.float16),
            "uwa": np.ascontiguousarray(
                uwS[:, :, 0:512].transpose(1, 0, 2).reshape(64, 2048)),
            "uwb": np.ascontiguousarray(
                uwS[:, :, 512:1024].transpose(1, 0, 2).reshape(64, 2048)),
        }
        in_maps.append(m)
    return in_maps


def kernel(x, u, A, B, stepNum):
    global LAST_RESULT
    stepNum = int(stepNum)
    nc = _get_nc()
    in_maps = _build_in_maps(x, u, A, B)
    res = run_bass_kernel_spmd(nc, in_maps, core_ids=list(range(NCORES)))
    LAST_RESULT = res
    out = np.empty((NCORES * BC, T, L), np.float32)
    for core in range(NCORES):
        od = np.asarray(res.results[core]["out"]).astype(np.float32)
        arr = od.reshape(2, S, 2, L, NQ, 16, BC)   # [h, m, rr, l, q, cl, b]
        arr = arr[:, :, ::-1]                      # k: 0 -> 2q+1, 1 -> 2q+2
        arr = arr.transpose(6, 0, 5, 1, 4, 2, 3)   # [b, h, cl, m, q, k, l]
        arr = np.ascontiguousarray(arr).reshape(BC, T, L)
        out[core * BC:(core + 1) * BC, 1:T, :] = arr[:, 0:T - 1, :]
    out[:, 0, :] = np.asarray(x, np.float32)[:, 0, :]
    if stepNum < T:
        out[:, stepNum:, :] = 0.0
    return out


# revision 23
# speedup vs baseline: 1.3397x; 1.0482x over previous
"""Trainium2 Bass kernel for the linear state rollout problem.

reference: s_{t+1} = A s_t + B u_t, resX[:, t, :] = s_t, s_0 = x[:, 0, :]
shapes: x [256, 1024, 64], u [256, 1024, 7], A [64, 64], B [64, 7]

Strategy (per core, data-parallel over batch: 32 batch elems / core):
  - 32 chunks of K=32 steps; column space 1024 = chunk*32 + batch.
  - All weights (A powers, pass-2 emit matrices W2, chunk-response
    matrices Vs, Kogge-Stone powers) precomputed on host in float64.
  - z_c (chunk input response) via 4 accumulating fp16 matmuls per
    column half — no sequential window chain.  s0 contribution
    (A^32 s0) host-computed, added on device.
  - Carries via 5-level Kogge-Stone scan over chunk columns (batched
    [64x64] fp32r matmuls + vector/gpsimd adds).  Chunks 0-15 are
    final after level 3, so pass-2 for the first column half overlaps
    level 4.
  - Pass 2: per 8-step window, 4 fp16 matmuls [128x128]@[128,512] emit
    2 states each (PSUM-emission-rate optimal); psum->sbuf fp16 copies
    split across vector/scalar/gpsimd; one 512KB output DMA per
    window (8 total) to amortize the ~600ns DMA trigger cost.
  - A few warmup matmuls at start ramp the PE DVFS p-state.
"""

import numpy as np

import concourse.bass as bass
import concourse.bacc as bacc
import concourse.tile as tile
from concourse import mybir
from concourse.bass_utils import run_bass_kernel_spmd

F32 = mybir.dt.float32
F32R = mybir.dt.float32r
FP16 = mybir.dt.float16

NCORES = 8
BC = 32          # batch per core
T = 1024
L = 64
CH = 7
NCHUNK = 32
S = 4            # window steps per chunk
R = 8            # timesteps per window
NQ = 4
NCOL = NCHUNK * BC   # 1024
NWARM = 6        # PE warmup matmuls

_NC_CACHE = None
LAST_RESULT = None


# ---------------------------------------------------------------- host math
def _build_weights(A, B):
    A = np.asarray(A, np.float64)
    B = np.asarray(B, np.float64)
    npow = {}

    def Ap(p):
        if p not in npow:
            npow[p] = np.linalg.matrix_power(A, p)
        return npow[p]

    TBrev = np.zeros((64, L), np.float64)
    for j in range(8):
        TBrev[8 * j:8 * j + CH, :] = (Ap(7 - j) @ B).T

    W2 = np.zeros((NQ, 128, 128), np.float64)
    for q in range(NQ):
        phi = 2 * q + 2
        plo = 2 * q + 1
        W2[q, 0:64, 0:64] = Ap(phi).T
        W2[q, 0:64, 64:128] = Ap(plo).T
        W2[q, 64:64 + 8 * phi, 0:64] = \
            TBrev[8 * (6 - 2 * q):8 * (6 - 2 * q) + 8 * phi]
        W2[q, 64:64 + 8 * plo, 64:128] = \
            TBrev[8 * (7 - 2 * q):8 * (7 - 2 * q) + 8 * plo]

    Vs = np.zeros((S, 64, L), np.float64)
    for s in range(S):
        for r in range(R):
            Vs[s, 8 * r:8 * r + CH, :] = (Ap(31 - 8 * s - r) @ B).T

    Q = np.stack([Ap(32 * (1 << j)).T for j in range(5)], axis=0)

    cz = np.zeros((64, 256), np.float16)
    for s in range(S):
        cz[:, 64 * s:64 * (s + 1)] = Vs[s].astype(np.float16)

    cw2 = np.zeros((128, 512), np.float16)
    for q in range(NQ):
        cw2[:, 128 * q:128 * (q + 1)] = W2[q].astype(np.float16)

    c32 = np.zeros((64, 320), np.float32)
    for j in range(5):
        c32[:, 64 * j:64 * (j + 1)] = Q[j].astype(np.float32)

    return cz, cw2, c32, Ap(32)


def _build_uw(uc):
    """uc [BC, T, CH] f32 -> [4, 64, 1024] fp16 window layout:
    col = c*32 + b, row = 8r + ch, block s, value u[b, 32c + 8s + r, ch]."""
    v = uc.reshape(BC, NCHUNK, S, R, CH).transpose(2, 3, 4, 1, 0)
    buf = np.zeros((S, R, 8, NCHUNK, BC), np.float16)
    buf[:, :, :CH, :, :] = v.astype(np.float16)
    return buf.reshape(S, 64, NCOL)


# ---------------------------------------------------------------- device
def _build_nc():
    nc = bacc.Bacc("TRN2", target_bir_lowering=False, debug=False,
                   num_devices=NCORES)

    uwa = nc.dram_tensor("uwa", [64, 2048], FP16, kind="ExternalInput").ap()
    uwb = nc.dram_tensor("uwb", [64, 2048], FP16, kind="ExternalInput").ap()
    cz = nc.dram_tensor("cz", [64, 256], FP16, kind="ExternalInput").ap()
    cw2 = nc.dram_tensor("cw2", [128, 512], FP16, kind="ExternalInput").ap()
    c32 = nc.dram_tensor("c32", [64, 320], F32R, kind="ExternalInput").ap()
    inj = nc.dram_tensor("inj", [64, 32], F32R, kind="ExternalInput").ap()
    s0h = nc.dram_tensor("s0h", [64, 32], FP16, kind="ExternalInput").ap()
    out = nc.dram_tensor("out", [2 * S, 128, 2048], FP16,
                         kind="ExternalOutput").ap()

    with tile.TileContext(nc) as tc:
        with (
            tc.tile_pool(name="const", bufs=1) as constp,
            tc.tile_pool(name="slab", bufs=1) as slabp,
            tc.tile_pool(name="stage", bufs=3) as stagep,
            tc.tile_pool(name="ps", bufs=7, space="PSUM") as psp,
        ):
            # ---- SBUF tiles
            zw = constp.tile([128, 256], FP16, tag="zw")
            w2 = constp.tile([128, 512], FP16, tag="w2")
            qb = constp.tile([64, 320], F32R, tag="qb")
            inj_sb = constp.tile([64, 32], F32R, tag="inj")
            slabs = slabp.tile([128, 4096], FP16, tag="slabs")
            zt = slabp.tile([64, NCOL], F32R, tag="zt")

            # ---- input DMAs.  Slab layout is h-major: block (h, s) at
            # cols h*2048 + s*512, so each half's u windows land in one
            # contiguous DMA.  scalar: weights (Vs first — gates z mms).
            nc.scalar.dma_start(out=zw[64:128, :], in_=cz)
            nc.sync.dma_start(out=slabs[64:128, 0:2048], in_=uwa)
            nc.sync.dma_start(out=slabs[64:128, 2048:4096], in_=uwb)
            nc.scalar.dma_start(out=qb[:], in_=c32)
            nc.scalar.dma_start(out=w2[:], in_=cw2)
            nc.scalar.dma_start(out=inj_sb[:], in_=inj)
            nc.scalar.dma_start(out=slabs[0:64, 0:32], in_=s0h)

            # ---- z: chunk responses, 4 accumulating matmuls per half
            psz = {}
            for h in range(2):
                psz[h] = psp.tile([64, 512], F32, tag="ps", name=f"psz{h}")
                for s in range(S):
                    nc.tensor.matmul(
                        psz[h][:, :],
                        zw[64:128, 64 * s:64 * (s + 1)],
                        slabs[64:128, h * 2048 + s * 512:h * 2048 + s * 512 + 512],
                        start=(s == 0), stop=(s == S - 1))
            nc.vector.tensor_copy(zt[:, 0:512], psz[0][:, :])
            nc.vector.tensor_add(zt[:, 0:32], zt[:, 0:32], inj_sb[:, :])
            nc.scalar.copy(zt[:, 512:1024], psz[1][:, :])

            # ---- Kogge-Stone over chunks, in place.  addA is split into
            # a main part (cols mmB never reads — keeps the A-chain
            # decoupled from the B-chain) and a boundary part that must
            # wait for mmB's read of the old values (WAR).
            def ks_level(j):
                d0 = 32 * (1 << j)
                cw = 512 - d0
                bnd = max(d0, 512 - d0)   # boundary start: [bnd, 512)
                psa = psp.tile([64, 512], F32, tag="ps", name=f"ksa{j}")
                nc.tensor.matmul(psa[:, 0:cw], qb[:, 64 * j:64 * (j + 1)],
                                 zt[:, 0:cw])
                psb = psp.tile([64, 512], F32, tag="ps", name=f"ksb{j}")
                nc.tensor.matmul(psb[:, :], qb[:, 64 * j:64 * (j + 1)],
                                 zt[:, 512 - d0:1024 - d0])
                if bnd > d0:
                    nc.vector.tensor_add(zt[:, d0:bnd],
                                         psa[:, 0:bnd - d0],
                                         zt[:, d0:bnd])
                nc.vector.tensor_add(zt[:, bnd:512],
                                     psa[:, bnd - d0:cw],
                                     zt[:, bnd:512])
                nc.vector.tensor_add(zt[:, 512:1024], psb[:, :],
                                     zt[:, 512:1024])

            # chunks 0-7 (cols 0:256) are final after level 2's main add,
            # so window (0,0)'s lower half can start before level 3.
            ks_level(0)
            ks_level(1)
            ks_level(2)
            nc.vector.tensor_copy(slabs[0:64, 32:256], zt[:, 0:224])
            ks_level(3)

            # ---- pass 2
            def pass2_step(h, m):
                st = stagep.tile([128, 2048], FP16, tag="st",
                                 name=f"st{h}{m}")
                base = h * 2048 + m * 512
                for q in (3, 0, 1, 2):
                    ps = psp.tile([128, 512], F32, tag="ps",
                                  name=f"q{h}{m}{q}")
                    nc.tensor.matmul(ps[:, :],
                                     w2[:, 128 * q:128 * (q + 1)],
                                     slabs[:, base:base + 512])
                    d = q * 512
                    if q == 3:
                        if m < S - 1:
                            nb = base + 512
                            nc.vector.tensor_copy(
                                slabs[0:64, nb:nb + 512], ps[0:64, :])
                            nc.scalar.copy(st[:, d:d + 512], ps[:, :])
                        else:
                            nc.scalar.copy(st[:, d:d + 512], ps[:, :])
                    elif q == 1:
                        nc.vector.tensor_copy(st[:, d:d + 512], ps[:, :])
                    elif q == 0:
                        nc.scalar.copy(st[:, d:d + 512], ps[:, :])
                    else:  # q == 2: split across both engines
                        if m < S - 1:
                            nc.vector.tensor_copy(st[:, d:d + 256],
                                                  ps[:, 0:256])
                            nc.scalar.copy(st[:, d + 256:d + 512],
                                           ps[:, 256:512])
                        else:
                            nc.vector.tensor_copy(st[:, d:d + 512],
                                                  ps[:, :])
                w = h * S + m
                if w == 2 * S - 1:
                    # split the last DMA so the tail transfer is shorter
                    nc.sync.dma_start(out=out[w, :, 0:1024],
                                      in_=st[:, 0:1024])
                    nc.sync.dma_start(out=out[w, :, 1024:2048],
                                      in_=st[:, 1024:2048])
                else:
                    nc.sync.dma_start(out=out[w], in_=st[:])

            # window (0,0) in column halves: lo fires off the A-main
            # chain; hi waits for the level-3 boundary add.
            st00 = stagep.tile([128, 2048], FP16, tag="st", name="st00")
            for half in range(2):
                c0 = half * 256
                if half == 1:
                    nc.vector.tensor_copy(slabs[0:64, 256:512],
                                          zt[:, 224:480])
                for q in (3, 0, 1, 2):
                    ps = psp.tile([128, 256], F32, tag="ps",
                                  name=f"q00{half}{q}")
                    nc.tensor.matmul(ps[:, :],
                                     w2[:, 128 * q:128 * (q + 1)],
                                     slabs[:, c0:c0 + 256])
                    d = q * 512 + c0
                    if q == 3:
                        nc.vector.tensor_copy(
                            slabs[0:64, 512 + c0:768 + c0], ps[0:64, :])
                        nc.scalar.copy(st00[:, d:d + 256], ps[:, :])
                    elif q == 0:
                        nc.scalar.copy(st00[:, d:d + 256], ps[:, :])
                    else:
                        nc.vector.tensor_copy(st00[:, d:d + 256],
                                              ps[:, :])
            nc.sync.dma_start(out=out[0], in_=st00[:])
            # level 4 (finalizes H1 carries) overlaps pass-2 H0
            ps4 = psp.tile([64, 512], F32, tag="ps", name="ks4")
            nc.tensor.matmul(ps4[:, :], qb[:, 256:320], zt[:, 0:512])
            nc.vector.tensor_add(zt[:, 512:1024], ps4[:, :],
                                 zt[:, 512:1024])
            pass2_step(0, 1)
            nc.gpsimd.tensor_copy(slabs[0:64, 2048:2560], zt[:, 480:992])
            pass2_step(0, 2)
            pass2_step(0, 3)
            for m in range(S):
                pass2_step(1, m)

    nc.compile()
    return nc


def _get_nc():
    global _NC_CACHE
    if _NC_CACHE is None:
        _NC_CACHE = _build_nc()
    return _NC_CACHE


def _build_in_maps(x, u, A, B):
    x = np.asarray(x, np.float32)
    u = np.asarray(u, np.float32)
    cz, cw2, c32, A32 = _build_weights(A, B)
    in_maps = []
    for core in range(NCORES):
        bsl = slice(core * BC, (core + 1) * BC)
        uwS = _build_uw(u[bsl])
        s0 = x[bsl, 0, :].T                       # [64, 32] f32
        m = {
            "cz": cz,
            "cw2": cw2,
            "c32": c32,
            "inj": (A32 @ s0.astype(np.float64)).astype(np.float32),
            "s0h": s0.astype(np.float16),
            "uwa": np.ascontiguousarray(
                uwS[:, :, 0:512].transpose(1, 0, 2).reshape(64, 2048)),
            "uwb": np.ascontiguousarray(
                uwS[:, :, 512:1024].transpose(1, 0, 2).reshape(64, 2048)),
        }
        in_maps.append(m)
    return in_maps


def kernel(x, u, A, B, stepNum):
    global LAST_RESULT
    stepNum = int(stepNum)
    nc = _get_nc()
    in_maps = _build_in_maps(x, u, A, B)
    res = run_bass_kernel_spmd(nc, in_maps, core_ids=list(range(NCORES)))
    LAST_RESULT = res
    out = np.empty((NCORES * BC, T, L), np.float32)
    for core in range(NCORES):
        od = np.asarray(res.results[core]["out"]).astype(np.float32)
        arr = od.reshape(2, S, 2, L, NQ, 16, BC)   # [h, m, rr, l, q, cl, b]
        arr = arr[:, :, ::-1]                      # k: 0 -> 2q+1, 1 -> 2q+2
        arr = arr.transpose(6, 0, 5, 1, 4, 2, 3)   # [b, h, cl, m, q, k, l]
        arr = np.ascontiguousarray(arr).reshape(BC, T, L)
        out[core * BC:(core + 1) * BC, 1:T, :] = arr[:, 0:T - 1, :]
    out[:, 0, :] = np.asarray(x, np.float32)[:, 0, :]
    if stepNum < T:
        out[:, stepNum:, :] = 0.0
    return out
